# revision 7
# baseline (speedup 1.0000x reference)
"""Causal self-attention with RoPE on 8 Trainium2 NeuronCores.

Problem: B=4, T=2048, D=1024, H=16, Hd=64 (fp32).

Sharding: tensor-parallel over heads — 2 heads per core, all 4 batches on
every core. Each core computes q/k/v projections for its 2 heads, RoPE,
causal softmax(QK^T)V, and a row-sharded slice of out_proj; the host sums
the 8 partial outputs (the tensor-parallel all-reduce done at gather).

Numerics: float32r (tf32) matmuls at full PE rate; fp32 PSUM accumulation.

Per core, per batch:
  1. QKV projection: stationary = 128-col tiles of x^T (host-supplied),
     streaming = wqkv^T [128, 384] -> q|k|v natural [t, d] in PSUM.
  2. RoPE cos/sin products on DVE in natural layout (free-dim half swaps);
     the final add is folded into the PE transpose by accumulating two
     transposes (cos-part + sin-part) into one PSUM tile -> qT,kT.
  3. Scores^T per (head, tq-chunk 512, tk-tile pair): two matmuls into one
     2-bank PSUM tile [128, 1024]; heads row-packed via base_partition
     0/64. One wide exp (ScalarE, scale folded). Causal diagonal pairs
     masked by DVE mask-multiply and gpsimd.affine_select, alternating.
  4. AV: lhsT = v|ones [128, 65] -> out^T psum [65, 512] per head; row 64 =
     softmax denominators. Reciprocal read straight from PSUM, gpsimd
     partition_broadcast, DVE multiply evicts normalized output.
  5. out_proj: lhsT = attn^T chunk (already transposed), rhs = w_o
     slice^T; two 512-wide matmuls into one 2-bank tile, single wide
     eviction, DMA out.
"""

import numpy as np
import concourse.bass as bass
import concourse.tile as tile
from concourse import bacc, mybir
from concourse.bass_utils import run_bass_kernel_spmd

F32 = mybir.dt.float32
F32R = mybir.dt.float32r
EXP = mybir.ActivationFunctionType.Exp
MULT = mybir.AluOpType.mult

B, T, D, H, HD = 4, 2048, 1024, 16, 64
NC_ = 8
TT = T // 128            # 16 token tiles
NCH = 4                  # tq chunks of 512
SCALE = 1.0 / np.sqrt(HD)


def to_tf32(a: np.ndarray) -> np.ndarray:
    u = np.ascontiguousarray(a, dtype=np.float32).view(np.uint32).astype(np.uint64)
    u = (u + 0xFFF + ((u >> 13) & 1)) & ~np.uint64(0x1FFF)
    return u.astype(np.uint32).view(np.float32)


def build_nc():
    nc = bacc.Bacc(None, target_bir_lowering=False)

    xt_d = nc.dram_tensor("xt", [B, 8, 128, T], F32R, kind="ExternalInput")
    wqkvt_d = nc.dram_tensor("wqkvt", [8, 128, 384], F32R, kind="ExternalInput")
    wot_d = nc.dram_tensor("wot", [128, 1024], F32R, kind="ExternalInput")
    cos_d = nc.dram_tensor("cosb", [128, TT, HD], F32, kind="ExternalInput")
    sin_d = nc.dram_tensor("sinb", [128, TT, HD], F32, kind="ExternalInput")
    ident_d = nc.dram_tensor("ident", [128, 128], F32R, kind="ExternalInput")
    ones_d = nc.dram_tensor("vones", [128, TT, 2, 1], F32R, kind="ExternalInput")
    out_d = nc.dram_tensor("out", [B, TT, 128, D], F32, kind="ExternalOutput")

    with tile.TileContext(nc) as tc:
        with (
            tc.tile_pool(name="const", bufs=1) as const,
            tc.tile_pool(name="xtp", bufs=1) as xtp,
            tc.tile_pool(name="qkt", bufs=2) as qkt,
            tc.tile_pool(name="vp", bufs=2) as vp,
            tc.tile_pool(name="attnp", bufs=2) as attnp,
            tc.tile_pool(name="rope", bufs=3) as rope,
            tc.tile_pool(name="expp", bufs=4) as expp,
            tc.tile_pool(name="outst", bufs=3) as outst,
            tc.tile_pool(name="smallp", bufs=2) as smallp,
            tc.tile_pool(name="ps", bufs=4, space="PSUM") as ps,
        ):
            w_sb = const.tile([128, 8, 384], F32R)
            wo_sb = const.tile([128, 1024], F32R)
            cos_sb = const.tile([128, TT, HD], F32)
            sin_sb = const.tile([128, TT, HD], F32)
            ident_sb = const.tile([128, 128], F32R)
            nc.sync.dma_start(w_sb[:], wqkvt_d[:].rearrange("c p n -> p c n"))
            nc.sync.dma_start(wo_sb[:], wot_d[:])
            nc.sync.dma_start(cos_sb[:], cos_d[:])
            nc.sync.dma_start(sin_sb[:], sin_d[:])
            nc.sync.dma_start(ident_sb[:], ident_d[:])

            for b in range(B):
                xt_sb = xtp.tile([128, 8, T], F32R, tag="xt", name="xt_sb")
                nc.sync.dma_start(xt_sb[:], xt_d[b].rearrange("c p t -> p c t"))

                # qkT_sb[:, 0, :] = qT, [:, 1, :] = kT (rows: 2 heads x 64 dims)
                qkT_sb = qkt.tile([128, 2, T], F32R, tag="qkT", name="qkT_sb")
                v_sb = vp.tile([128, TT, 2, 65], F32R, tag="v", name="v_sb")
                attn_sb = attnp.tile([128, T], F32R, tag="attn", name="attn_sb")
                nc.sync.dma_start(v_sb[:, :, :, 64:65], ones_d[:])

                # --- projection + rope + transpose, per 128-token tile ---
                for tt in range(TT):
                    pq = ps.tile([128, 1024], F32, tag="ps", name="pq")
                    for dc in range(8):
                        nc.tensor.matmul(
                            pq[:, 0:384],
                            xt_sb[:, dc, tt * 128:(tt + 1) * 128],
                            w_sb[:, dc, :],
                            start=(dc == 0), stop=(dc == 7),
                        )
                    # rope products: qkc = (q|k)*cos ; qks = swap(q|k)*sin'
                    qkc = rope.tile([128, 256], F32R, tag="ropec", name="qkc")
                    qks = rope.tile([128, 256], F32R, tag="ropes", name="qks")
                    pq4 = pq[:, 0:256].rearrange("p (b d) -> p b d", d=HD)
                    qkc4 = qkc[:].rearrange("p (b d) -> p b d", d=HD)
                    qks4 = qks[:].rearrange("p (b d) -> p b d", d=HD)
                    cos_bc = cos_sb[:, tt, None, :].to_broadcast((128, 4, HD))
                    sin_bc = sin_sb[:, tt, None, :].to_broadcast((128, 4, HD))
                    nc.vector.tensor_tensor(qkc4, pq4, cos_bc, MULT)
                    nc.vector.tensor_tensor(
                        qks4[:, :, 0:32], pq4[:, :, 32:64],
                        sin_bc[:, :, 0:32], MULT)
                    nc.vector.tensor_tensor(
                        qks4[:, :, 32:64], pq4[:, :, 0:32],
                        sin_bc[:, :, 32:64], MULT)
                    # v eviction (rounds to f32r)
                    nc.vector.tensor_copy(
                        v_sb[:, tt, :, 0:64],
                        pq[:, 256:384].rearrange("p (h d) -> p h d", d=HD))
                    # (kept on DVE: gpsimd cannot read PSUM)
                    # transposes with rope-add folded in: psum accumulates
                    # transpose(qkc) + transpose(qks). q -> bank0, k -> bank1.
                    tr = ps.tile([128, 1024], F32R, tag="ps", name="tr")
                    for half in (0, 1):
                        o = tr[:, 512 * half:512 * half + 128]
                        nc.tensor.matmul(
                            o, qkc[:, 128 * half:128 * half + 128], ident_sb[:],
                            is_transpose=True, start=True, stop=False)
                        nc.tensor.matmul(
                            o, qks[:, 128 * half:128 * half + 128], ident_sb[:],
                            is_transpose=True, start=False, stop=True)
                    nc.vector.tensor_copy(
                        qkT_sb[:, :, tt * 128:(tt + 1) * 128],
                        tr[:].rearrange("p (a x) -> p a x", x=512)[:, :, 0:128])

                # --- attention per tq chunk of 512 ---
                qT = qkT_sb[:, 0, :]
                kT = qkT_sb[:, 1, :]
                for c in range(NCH):
                    njt = 4 * c + 4
                    avd = ps.tile([128, 1024], F32, tag="ps", name="avd")
                    # units: (jp, h) staggered so PE stays ahead of ACT/DVE
                    units = [(jp, h) for jp in range(njt // 2) for h in (0, 1)]
                    stq, exq = {}, {}

                    def emit_sc(u, c=c, stq=stq):
                        jp, h = u
                        st = ps.tile([128, 1024], F32, tag="ps", name="st")
                        stq[u] = st
                        for idx in (0, 1):
                            j = 2 * jp + idx
                            nc.tensor.matmul(
                                st[:, 512 * idx:512 * idx + 512],
                                kT[64 * h:64 * h + 64, j * 128:(j + 1) * 128],
                                qT[64 * h:64 * h + 64, c * 512:(c + 1) * 512],
                                start=True, stop=True)

                    def emit_ex(u, c=c, stq=stq, exq=exq):
                        jp, h = u
                        ex = expp.tile([128, 2, 512], F32R, tag="ex", name="ex")
                        exq[u] = ex
                        st2 = stq[u][:].rearrange("p (a x) -> p a x", x=512)
                        diag = 2 * jp >= 4 * c
                        v = jp - 2 * c if diag else 0
                        q0 = 256 * v if diag else 0
                        nc.scalar.activation(
                            ex[:, :, q0:512], st2[:, :, q0:512],
                            EXP, scale=float(SCALE))
                        if diag:
                            # triangular strips at q in [256v+128hh, +128):
                            # condition reduces to s >= p uniformly
                            e0 = ex[:]
                            strips = bass.AP(
                                e0.tensor, e0.offset + 256 * v,
                                [e0.ap[0], [640, 2], [1, 128]])
                            nc.gpsimd.affine_select(
                                out=strips, in_=strips,
                                compare_op=mybir.AluOpType.is_ge,
                                fill=0.0,
                                base=0,
                                channel_multiplier=-1,
                                pattern=[[0, 2], [1, 128]])

                    def emit_av(u, c=c, njt=njt, exq=exq):
                        jp, h = u
                        ex = exq.pop(u)
                        for idx in (0, 1):
                            j = 2 * jp + idx
                            q0 = 128 * (j - 4 * c) if 2 * jp >= 4 * c else 0
                            nc.tensor.matmul(
                                avd[0:65, 512 * h + q0:512 * h + 512],
                                v_sb[:, j, h, :],
                                ex[:, idx, q0:512],
                                start=(j == 0), stop=(j == njt - 1),
                                skip_group_check=True)

                    n = len(units)
                    for i in range(n + 2):
                        if i < n:
                            emit_sc(units[i])
                        if 0 <= i - 1 < n:
                            emit_ex(units[i - 1])
                        if 0 <= i - 2 < n:
                            emit_av(units[i - 2])

                    rec = smallp.tile([1, 1024], F32, tag="rec", name="rec")
                    bc = smallp.tile([64, 1024], F32, tag="bc", name="bc")
                    nc.vector.reciprocal(rec[0:1, :], avd[64:65, :])
                    nc.gpsimd.partition_broadcast(bc[:], rec[0:1, :])
                    for h in (0, 1):
                        nc.vector.tensor_tensor(
                            attn_sb[64 * h:64 * h + 64, c * 512:(c + 1) * 512],
                            avd[0:64, 512 * h:512 * h + 512],
                            bc[:, 512 * h:512 * h + 512], MULT)

                # --- out projection (partial over this core's 128 dims) ---
                for tt in range(TT):
                    po = ps.tile([128, 1024], F32, tag="ps", name="po")
                    for ch in (0, 1):
                        nc.tensor.matmul(
                            po[:, 512 * ch:512 * ch + 512],
                            attn_sb[:, tt * 128:(tt + 1) * 128],
                            wo_sb[:, 512 * ch:512 * ch + 512],
                            start=True, stop=True)
                    ost = outst.tile([128, 1024], F32, tag="ost", name="ost")
                    if tt % 4 == 0:
                        nc.scalar.copy(ost[:], po[:])
                    else:
                        nc.vector.tensor_copy(ost[:], po[:])
                    nc.sync.dma_start(out_d[b, tt, :, :], ost[:])

    nc.compile()
    return nc


_NC_CACHE = None


def get_nc():
    global _NC_CACHE
    if _NC_CACHE is None:
        _NC_CACHE = build_nc()
    return _NC_CACHE


def prep_in_maps(x, cos, sin, w_q, w_k, w_v, w_o):
    """Host-side sharding: returns per-core input dicts."""
    x = np.asarray(x, np.float32)
    cos = np.asarray(cos, np.float32)
    sin = np.asarray(sin, np.float32)

    xt = to_tf32(np.ascontiguousarray(x.transpose(0, 2, 1)).reshape(B, 8, 128, T))

    cosb = np.ascontiguousarray(cos.reshape(TT, 128, HD).transpose(1, 0, 2))
    sinneg = sin.copy()
    sinneg[:, 0:HD // 2] *= -1.0
    sinb = np.ascontiguousarray(sinneg.reshape(TT, 128, HD).transpose(1, 0, 2))

    ident = np.eye(128, dtype=np.float32)
    vones = np.ones((128, TT, 2, 1), np.float32)

    in_maps = []
    for c in range(NC_):
        rows = slice(128 * c, 128 * (c + 1))
        wqkv = np.concatenate([w_q[rows], w_k[rows], w_v[rows]], axis=0)
        wqkvt = to_tf32(np.ascontiguousarray(wqkv.T).reshape(8, 128, 384))
        wot = to_tf32(np.ascontiguousarray(w_o[:, rows].T))
        in_maps.append({
            "xt": xt, "wqkvt": wqkvt, "wot": wot,
            "cosb": cosb, "sinb": sinb, "ident": ident, "vones": vones,
        })
    return in_maps


def postprocess(results):
    out = np.zeros((B, TT, 128, D), np.float64)
    for r in results:
        out += r["out"].astype(np.float64)
    return out.reshape(B, T, D).astype(np.float32)


def kernel(x, cos, sin, w_q, w_k, w_v, w_o):
    nc = get_nc()
    in_maps = prep_in_maps(x, cos, sin, w_q, w_k, w_v, w_o)
    res = run_bass_kernel_spmd(nc, in_maps, core_ids=list(range(NC_)),
                               trace=False)
    return postprocess(res.results)


# revision 8
# speedup vs baseline: 72.0513x; 72.0513x over previous
"""Causal self-attention with RoPE on 8 Trainium2 NeuronCores.

Problem: B=4, T=2048, D=1024, H=16, Hd=64 (fp32).

Sharding: tensor-parallel over heads — 2 heads per core, all 4 batches on
every core. Each core computes q/k/v projections for its 2 heads, RoPE,
causal softmax(QK^T)V, and a row-sharded slice of out_proj; the host sums
the 8 partial outputs (the tensor-parallel all-reduce done at gather).

Numerics: float32r (tf32) matmuls at full PE rate; fp32 PSUM accumulation.

Per core, per batch:
  1. QKV projection (software-pipelined over 128-token tiles): stationary =
     128-col tiles of x^T (host-supplied), streaming = wqkv^T [128, 384]
     -> q|k|v natural [t, d] in PSUM.
  2. RoPE cos/sin products on DVE in natural layout (free-dim half swaps);
     the rope add is folded into the PE transpose by accumulating the two
     transposes (cos part + sin part) into one PSUM region -> qT|kT.
  3. Scores^T per (head, tq-chunk 512, tk-tile pair): two matmuls into one
     2-bank PSUM tile [128, 1024]; the two heads ride concurrent row
     groups (base_partition 0/64). One wide exp on ScalarE with the
     1/sqrt(Hd) scale folded in, narrowed to the causally live region on
     diagonal pairs. Diagonal triangles cleared by a single
     gpsimd.affine_select over both 128-wide strips (condition s >= p).
  4. AV: lhsT = v|ones [128, 65] -> out^T psum [65, 512] per head; psum
     row 64 accumulates the softmax denominators for free. Per chunk: one
     wide reciprocal straight from PSUM, one gpsimd partition_broadcast,
     DVE multiplies evict the normalized output (division deferred out of
     the matmul chain).
  5. out_proj interleaved after each chunk: lhsT = attn^T tile (already
     transposed), rhs = w_o slice^T; two 512-wide matmuls into one 2-bank
     tile, one wide eviction alternating ScalarE/DVE, DMA out.
"""

import numpy as np
import concourse.bass as bass
import concourse.tile as tile
from concourse import bacc, mybir
from concourse.bass_utils import run_bass_kernel_spmd

F32 = mybir.dt.float32
F32R = mybir.dt.float32r
EXP = mybir.ActivationFunctionType.Exp
MULT = mybir.AluOpType.mult

B, T, D, H, HD = 4, 2048, 1024, 16, 64
NC_ = 8
TT = T // 128
NCH = 4
SCALE = 1.0 / np.sqrt(HD)


def to_tf32(a: np.ndarray) -> np.ndarray:
    u = np.ascontiguousarray(a, dtype=np.float32).view(np.uint32).astype(np.uint64)
    u = (u + 0xFFF + ((u >> 13) & 1)) & ~np.uint64(0x1FFF)
    return u.astype(np.uint32).view(np.float32)


def build_nc():
    nc = bacc.Bacc(None, target_bir_lowering=False)

    xt_d = nc.dram_tensor("xt", [B, 8, 128, T], F32R, kind="ExternalInput")
    wqkvt_d = nc.dram_tensor("wqkvt", [8, 128, 384], F32R, kind="ExternalInput")
    wot_d = nc.dram_tensor("wot", [128, 1024], F32R, kind="ExternalInput")
    cos_d = nc.dram_tensor("cosb", [128, TT, HD], F32, kind="ExternalInput")
    sin_d = nc.dram_tensor("sinb", [128, TT, HD], F32, kind="ExternalInput")
    ident_d = nc.dram_tensor("ident", [128, 128], F32R, kind="ExternalInput")
    ones_d = nc.dram_tensor("vones", [128, TT, 2, 1], F32R, kind="ExternalInput")
    out_d = nc.dram_tensor("out", [B, TT, 128, D], F32, kind="ExternalOutput")

    with tile.TileContext(nc) as tc:
        with (
            tc.tile_pool(name="const", bufs=1) as const,
            tc.tile_pool(name="xtp", bufs=1) as xtp,
            tc.tile_pool(name="qkt", bufs=2) as qkt,
            tc.tile_pool(name="vp", bufs=2) as vp,
            tc.tile_pool(name="attnp", bufs=2) as attnp,
            tc.tile_pool(name="rope", bufs=3) as rope,
            tc.tile_pool(name="expp", bufs=3) as expp,
            tc.tile_pool(name="outst", bufs=3) as outst,
            tc.tile_pool(name="smallp", bufs=2) as smallp,
            tc.tile_pool(name="ps", bufs=4, space="PSUM") as ps,
        ):
            w_sb = const.tile([128, 8, 384], F32R)
            wo_sb = const.tile([128, 1024], F32R)
            cos_sb = const.tile([128, TT, HD], F32)
            sin_sb = const.tile([128, TT, HD], F32)
            ident_sb = const.tile([128, 128], F32R)
            nc.sync.dma_start(w_sb[:], wqkvt_d[:].rearrange("c p n -> p c n"))
            nc.sync.dma_start(wo_sb[:], wot_d[:])
            nc.sync.dma_start(cos_sb[:], cos_d[:])
            nc.sync.dma_start(sin_sb[:], sin_d[:])
            nc.sync.dma_start(ident_sb[:], ident_d[:])

            for b in range(B):
                xt_sb = xtp.tile([128, 8, T], F32R, tag="xt", name="xt_sb")
                nc.sync.dma_start(xt_sb[:], xt_d[b].rearrange("c p t -> p c t"))

                qkT_sb = qkt.tile([128, 2, T], F32R, tag="qkT", name="qkT_sb")
                v_sb = vp.tile([128, TT, 2, 65], F32R, tag="v", name="v_sb")
                attn_sb = attnp.tile([128, T], F32R, tag="attn", name="attn_sb")
                nc.sync.dma_start(v_sb[:, :, :, 64:65], ones_d[:])

                # --- projection + rope + transpose, software-pipelined ---
                pqs, ropes = {}, {}

                def emit_proj(tt, xt_sb=xt_sb):
                    pq = ps.tile([128, 1024], F32, tag="ps", name="pq")
                    pqs[tt] = pq
                    for dc in range(8):
                        nc.tensor.matmul(
                            pq[:, 0:384],
                            xt_sb[:, dc, tt * 128:(tt + 1) * 128],
                            w_sb[:, dc, :],
                            start=(dc == 0), stop=(dc == 7))

                def emit_rope(tt, v_sb=v_sb):
                    pq = pqs.pop(tt)
                    qkc = rope.tile([128, 256], F32R, tag="ropec", name="qkc")
                    qks = rope.tile([128, 256], F32R, tag="ropes", name="qks")
                    ropes[tt] = (qkc, qks)
                    pq4 = pq[:, 0:256].rearrange("p (b d) -> p b d", d=HD)
                    qkc4 = qkc[:].rearrange("p (b d) -> p b d", d=HD)
                    qks4 = qks[:].rearrange("p (b d) -> p b d", d=HD)
                    cos_bc = cos_sb[:, tt, None, :].to_broadcast((128, 4, HD))
                    sin_bc = sin_sb[:, tt, None, :].to_broadcast((128, 4, HD))
                    nc.vector.tensor_tensor(qkc4, pq4, cos_bc, MULT)
                    nc.vector.tensor_tensor(
                        qks4[:, :, 0:32], pq4[:, :, 32:64],
                        sin_bc[:, :, 0:32], MULT)
                    nc.vector.tensor_tensor(
                        qks4[:, :, 32:64], pq4[:, :, 0:32],
                        sin_bc[:, :, 32:64], MULT)
                    nc.vector.tensor_copy(
                        v_sb[:, tt, :, 0:64],
                        pq[:, 256:384].rearrange("p (h d) -> p h d", d=HD))

                def emit_tr(tt, qkT_sb=qkT_sb):
                    qkc, qks = ropes.pop(tt)
                    tr = ps.tile([128, 1024], F32R, tag="ps", name="tr")
                    for half in (0, 1):
                        o = tr[:, 512 * half:512 * half + 128]
                        nc.tensor.matmul(
                            o, qkc[:, 128 * half:128 * half + 128], ident_sb[:],
                            is_transpose=True, start=True, stop=False)
                        nc.tensor.matmul(
                            o, qks[:, 128 * half:128 * half + 128], ident_sb[:],
                            is_transpose=True, start=False, stop=True)
                    nc.vector.tensor_copy(
                        qkT_sb[:, :, tt * 128:(tt + 1) * 128],
                        tr[:].rearrange("p (a x) -> p a x", x=512)[:, :, 0:128])

                for tt in range(TT):
                    emit_proj(tt)
                    emit_rope(tt)
                    if tt >= 1:
                        emit_tr(tt - 1)
                emit_tr(TT - 1)

                # --- attention, out_proj interleaved per tq chunk ---
                qT = qkT_sb[:, 0, :]
                kT = qkT_sb[:, 1, :]
                for c in range(NCH):
                    njt = 4 * c + 4
                    avd = ps.tile([128, 1024], F32, tag="ps", name="avd")
                    units = [(jp, h) for jp in range(njt // 2) for h in (0, 1)]
                    stq, exq = {}, {}

                    def emit_sc(u, c=c, stq=stq, qT=qT, kT=kT):
                        jp, h = u
                        st = ps.tile([128, 1024], F32, tag="ps", name="st")
                        stq[u] = st
                        for idx in (0, 1):
                            j = 2 * jp + idx
                            nc.tensor.matmul(
                                st[:, 512 * idx:512 * idx + 512],
                                kT[64 * h:64 * h + 64, j * 128:(j + 1) * 128],
                                qT[64 * h:64 * h + 64, c * 512:(c + 1) * 512],
                                start=True, stop=True)

                    def emit_ex(u, c=c, stq=stq, exq=exq):
                        jp, h = u
                        ex = expp.tile([128, 2, 512], F32R, tag="ex", name="ex")
                        exq[u] = ex
                        st2 = stq.pop(u)[:].rearrange("p (a x) -> p a x", x=512)
                        diag = 2 * jp >= 4 * c
                        v = jp - 2 * c if diag else 0
                        q0 = 256 * v if diag else 0
                        nc.scalar.activation(
                            ex[:, :, q0:512], st2[:, :, q0:512],
                            EXP, scale=float(SCALE))
                        if diag:
                            # triangular strips at q in [256v+128hh, +128):
                            # the causal condition reduces to s >= p
                            e0 = ex[:]
                            strips = bass.AP(
                                e0.tensor, e0.offset + 256 * v,
                                [e0.ap[0], [640, 2], [1, 128]])
                            nc.gpsimd.affine_select(
                                out=strips, in_=strips,
                                compare_op=mybir.AluOpType.is_ge,
                                fill=0.0, base=0,
                                channel_multiplier=-1,
                                pattern=[[0, 2], [1, 128]])

                    def emit_av(u, c=c, njt=njt, avd=avd, exq=exq, v_sb=v_sb):
                        jp, h = u
                        ex = exq.pop(u)
                        for idx in (0, 1):
                            j = 2 * jp + idx
                            q0 = 128 * (j - 4 * c) if 2 * jp >= 4 * c else 0
                            nc.tensor.matmul(
                                avd[0:65, 512 * h + q0:512 * h + 512],
                                v_sb[:, j, h, :],
                                ex[:, idx, q0:512],
                                start=(j == 0), stop=(j == njt - 1),
                                skip_group_check=True)

                    n = len(units)
                    for i in range(n + 2):
                        if i < n:
                            emit_sc(units[i])
                        if 0 <= i - 1 < n:
                            emit_ex(units[i - 1])
                        if 0 <= i - 2 < n:
                            emit_av(units[i - 2])

                    rec = smallp.tile([1, 1024], F32, tag="rec", name="rec")
                    bc = smallp.tile([64, 1024], F32, tag="bc", name="bc")
                    nc.vector.reciprocal(rec[0:1, :], avd[64:65, :])
                    nc.gpsimd.partition_broadcast(bc[:], rec[0:1, :])
                    for h in (0, 1):
                        nc.vector.tensor_tensor(
                            attn_sb[64 * h:64 * h + 64, c * 512:(c + 1) * 512],
                            avd[0:64, 512 * h:512 * h + 512],
                            bc[:, 512 * h:512 * h + 512], MULT)

                    # out_proj for the token tiles this chunk completed
                    for tt in range(4 * c, 4 * c + 4):
                        po = ps.tile([128, 1024], F32, tag="ps", name="po")
                        for ch in (0, 1):
                            nc.tensor.matmul(
                                po[:, 512 * ch:512 * ch + 512],
                                attn_sb[:, tt * 128:(tt + 1) * 128],
                                wo_sb[:, 512 * ch:512 * ch + 512],
                                start=True, stop=True)
                        ost = outst.tile([128, 1024], F32, tag="ost", name="ost")
                        if tt % 2 == 0:
                            nc.scalar.copy(ost[:], po[:])
                        else:
                            nc.vector.tensor_copy(ost[:], po[:])
                        nc.sync.dma_start(out_d[b, tt, :, :], ost[:])

    nc.compile()
    return nc


_NC_CACHE = None


def get_nc():
    global _NC_CACHE
    if _NC_CACHE is None:
        _NC_CACHE = build_nc()
    return _NC_CACHE


def prep_in_maps(x, cos, sin, w_q, w_k, w_v, w_o):
    """Host-side sharding: returns per-core input dicts."""
    x = np.asarray(x, np.float32)
    cos = np.asarray(cos, np.float32)
    sin = np.asarray(sin, np.float32)

    xt = to_tf32(np.ascontiguousarray(x.transpose(0, 2, 1)).reshape(B, 8, 128, T))

    cosb = np.ascontiguousarray(cos.reshape(TT, 128, HD).transpose(1, 0, 2))
    sinneg = sin.copy()
    sinneg[:, 0:HD // 2] *= -1.0
    sinb = np.ascontiguousarray(sinneg.reshape(TT, 128, HD).transpose(1, 0, 2))

    ident = np.eye(128, dtype=np.float32)
    vones = np.ones((128, TT, 2, 1), np.float32)

    in_maps = []
    for c in range(NC_):
        rows = slice(128 * c, 128 * (c + 1))
        wqkv = np.concatenate([w_q[rows], w_k[rows], w_v[rows]], axis=0)
        wqkvt = to_tf32(np.ascontiguousarray(wqkv.T).reshape(8, 128, 384))
        wot = to_tf32(np.ascontiguousarray(w_o[:, rows].T))
        in_maps.append({
            "xt": xt, "wqkvt": wqkvt, "wot": wot,
            "cosb": cosb, "sinb": sinb, "ident": ident, "vones": vones,
        })
    return in_maps


def postprocess(results):
    out = np.zeros((B, TT, 128, D), np.float64)
    for r in results:
        out += r["out"].astype(np.float64)
    return out.reshape(B, T, D).astype(np.float32)


def kernel(x, cos, sin, w_q, w_k, w_v, w_o):
    nc = get_nc()
    in_maps = prep_in_maps(x, cos, sin, w_q, w_k, w_v, w_o)
    res = run_bass_kernel_spmd(nc, in_maps, core_ids=list(range(NC_)),
                               trace=False)
    return postprocess(res.results)
